# revision 25
# baseline (speedup 1.0000x reference)
"""Trainium2 Bass kernel for nn_MultiHeadAttention_63015760167496.

Computation (see reference): qkv = x @ Wqkv; RoPE on q,k; causal softmax
attention per head; out = einsum('bhts,bshd->bhtd', probs, v);
out.reshape(B,T,C) @ Wout  -- NOTE the reshape is a *head-major* flatten of
[B,H,T,D] into [B,T,C], so final-output row r = h*128 + t//16 depends only on
head h.  Sharding: head-parallel over 8 cores (2 heads/core); every core
computes its two heads end-to-end and produces final-output rows
[256*i, 256*i+256).  Host concatenates -- no collectives.

All matmul operands are bf16 (fp32 PSUM accumulation): bf16 weight loads use
FWL so LDWEIGHTS hides under matmul streaming, and DMA traffic halves vs
fp32.  Attention runs in S^T layout ([s,t]) and is interleaved into the QKV
projection stream: attention t-tile `ta` only needs tokens < 512*(ta+1), so
it is emitted right after projection chunk `ta`, letting the scalar-engine
exp (the attention pacer) overlap projection matmuls on the PE.  Softmax
denominator via ones-vector matmul accumulated in PSUM over the s-loop;
normalization via a K=1 broadcast matmul.  No running max is needed (scores
are O(5), fp32 psum).
"""

import math
import sys

for _p in ("/opt/trn_rl_repo", "/root/.axon_site/_ro/trn_rl_repo"):
    if _p not in sys.path:
        sys.path.insert(0, _p)

import numpy as np
import ml_dtypes

import concourse.bass as bass
import concourse.mybir as mybir
import concourse.tile as tile
from concourse import bacc
from concourse.bass_utils import run_bass_kernel_spmd

B, T, C = 2, 2048, 2048
H = 16            # heads total
D = C // H        # 128 head dim
HALF = D // 2     # 64
P = 128
KO = C // P       # 16 contraction chunks
NCORES = 8
HPC = H // NCORES  # 2 heads per core
TQ = 512          # t-tile for qkv projection == attention t-tile
NT = T // TQ      # 4
TA = 512
NTA = T // TA     # 4
NSC = T // P      # 16 s-chunks
TC = 512          # col-tile for output projection
ROPE_BASE = 10000.0
SCALE = 1.0 / math.sqrt(D)

f32 = mybir.dt.float32
f32r = mybir.dt.float32r
bf16 = mybir.dt.bfloat16
nbf16 = ml_dtypes.bfloat16


def _build():
    nc = bacc.Bacc("TRN2", target_bir_lowering=False, debug=False,
                   num_devices=NCORES)

    # host-pre-tiled x^T: xTt[b, ti, p, ko, u] = x[b, ti*TQ+u, ko*128+p]
    xTt = nc.dram_tensor("xTt", [B, NT, P, KO, TQ], bf16, kind="ExternalInput")
    # host-pre-chunked weights: w[p, ko, m] = W[ko*128+p, m]
    wq = nc.dram_tensor("wq", [P, KO, HPC * D], bf16, kind="ExternalInput")
    wk = nc.dram_tensor("wk", [P, KO, HPC * D], bf16, kind="ExternalInput")
    wv = nc.dram_tensor("wv", [P, KO, HPC * D], bf16, kind="ExternalInput")
    wout = nc.dram_tensor("wout", [P, KO, C], bf16, kind="ExternalInput")
    cs2 = nc.dram_tensor("cs2", [P, T], bf16, kind="ExternalInput")  # [cos;cos]
    sn2 = nc.dram_tensor("sn2", [P, T], bf16, kind="ExternalInput")  # [sin;sin]
    maskM = nc.dram_tensor("maskM", [P, P], bf16, kind="ExternalInput")
    y = nc.dram_tensor("y", [B, HPC * D, C], f32, kind="ExternalOutput")

    with tile.TileContext(nc) as tc:
        with tc.tile_pool(name="const", bufs=1) as cp_, \
             tc.tile_pool(name="qkv", bufs=1) as qp, \
             tc.tile_pool(name="ot", bufs=1) as op_, \
             tc.tile_pool(name="small", bufs=2) as sp:

            wq_sb = cp_.tile([P, KO, HPC * D], bf16, tag="wq")
            wk_sb = cp_.tile([P, KO, HPC * D], bf16, tag="wk")
            wv_sb = cp_.tile([P, KO, HPC * D], bf16, tag="wv")
            wout_sb = cp_.tile([P, KO, C], bf16, tag="wout")
            cs_sb = cp_.tile([P, T], bf16, tag="cs")
            sn_sb = cp_.tile([P, T], bf16, tag="sn")
            mask_sb = cp_.tile([P, P], bf16, tag="mask")
            ones_f1 = cp_.tile([1, P], f32, tag="ones_f1")
            nc.vector.memset(ones_f1[:], 1.0)
            ones_rowb = cp_.tile([1, P], bf16, tag="ones_rowb")
            nc.vector.tensor_copy(ones_rowb[:], ones_f1[:])
            ones_f32 = cp_.tile([P, 1], f32, tag="ones_f32")
            nc.vector.memset(ones_f32[:], 1.0)
            ones_col = cp_.tile([P, 1], bf16, tag="ones_col")
            nc.vector.tensor_copy(ones_col[:], ones_f32[:])

            # warm up the PE (HAM clock gate) with dummy matmuls on a
            # memset tile while the first DMAs land
            warm = cp_.tile([P, TC], bf16, tag="warm")
            nc.vector.memset(warm[:], 0.0)

            # persistent attention outputs O^T per (b, local head): [d, t]
            oT = [[op_.tile([P, T], bf16, tag=f"oT{b}{hh}", name=f"oT{b}{hh}")
                   for hh in range(HPC)] for b in range(B)]

            for b in range(B):
                qT = [qp.tile([P, T], bf16, tag=f"qT{hh}", name=f"qT{b}{hh}")
                      for hh in range(HPC)]
                kT = [qp.tile([P, T], bf16, tag=f"kT{hh}", name=f"kT{b}{hh}")
                      for hh in range(HPC)]
                # both heads interleaved: vt[:, s, hh*D:(hh+1)*D]
                vt = qp.tile([P, NSC, HPC * D], bf16, tag="vt", name=f"v{b}")

                with tc.tile_pool(name=f"xt{b}", bufs=2) as xp, \
                     tc.tile_pool(name=f"psW{b}", bufs=5, space="PSUM") as psw, \
                     tc.tile_pool(name=f"psO{b}", bufs=2, space="PSUM") as pso, \
                     tc.tile_pool(name=f"psSum{b}", bufs=1, space="PSUM") as pssum, \
                     tc.tile_pool(name=f"rope{b}", bufs=2) as rp, \
                     tc.tile_pool(name=f"pt{b}", bufs=3) as ptp:

                    def rope(ps, dst, sl, cs, sn):
                        # tcos = ps * [cos;cos]; tsw pre-swaps halves against
                        # sn = [-sin;+sin] (sign baked into the table), so one
                        # full-width aligned add finishes the rotation.  The
                        # partition-shifted reads sit on the PSUM-input muls,
                        # which the SB/SB base-partition rule exempts.
                        tcos = rp.tile([P, TQ], f32, tag="tcos")
                        tsw = rp.tile([P, TQ], f32, tag="tsw")
                        nc.vector.tensor_mul(tcos[:], ps[:], cs)
                        nc.vector.tensor_mul(tsw[0:HALF, :],
                                             ps[HALF:P, :], sn[0:HALF, :])
                        nc.vector.tensor_mul(tsw[HALF:P, :],
                                             ps[0:HALF, :], sn[HALF:P, :])
                        nc.gpsimd.tensor_add(dst[:, sl], tcos[:], tsw[:])

                    def qkv_chunk(ti):
                        sl = slice(ti * TQ, (ti + 1) * TQ)
                        xt = xp.tile([P, KO, TQ], bf16, tag="xt")
                        if b == 0 and ti == 0:
                            # split first loads so matmuls start sooner, and
                            # stagger the other const loads behind them
                            nc.sync.dma_start(wq_sb[:, 0:KO // 2],
                                              wq.ap()[:, 0:KO // 2])
                            nc.sync.dma_start(xt[:, 0:KO // 2, :],
                                              xTt.ap()[b, ti, :, 0:KO // 2])
                            nc.sync.dma_start(wq_sb[:, KO // 2:KO],
                                              wq.ap()[:, KO // 2:KO])
                            nc.sync.dma_start(xt[:, KO // 2:KO, :],
                                              xTt.ap()[b, ti, :, KO // 2:KO])
                            nc.sync.dma_start(wk_sb[:], wk.ap())
                            nc.sync.dma_start(wv_sb[:], wv.ap())
                            nc.sync.dma_start(cs_sb[:], cs2.ap())
                            nc.sync.dma_start(sn_sb[:], sn2.ap())
                            nc.sync.dma_start(mask_sb[:], maskM.ap())
                        else:
                            nc.sync.dma_start(xt[:], xTt.ap()[b, ti])
                        cs = cs_sb[:, sl]
                        sn = sn_sb[:, sl]
                        for hh in range(HPC):
                            hsl = slice(hh * D, (hh + 1) * D)
                            for w_sb, dst in ((wq_sb, qT[hh]), (wk_sb, kT[hh])):
                                ps = psqk.tile([P, TQ], f32, tag="qk")
                                for ko in range(KO):
                                    nc.tensor.matmul(ps[:], w_sb[:, ko, hsl],
                                                     xt[:, ko, :],
                                                     start=(ko == 0),
                                                     stop=(ko == KO - 1))
                                rope(ps, dst, sl, cs, sn)
                        for pair in range(TQ // P // 2):
                            # two 128-token sub-tiles share one PSUM bank so
                            # a single [P, 512] copy drains both
                            psv = psvp.tile([P, 2, HPC * D], f32, tag="v")
                            for half in range(2):
                                sub = pair * 2 + half
                                for ko in range(KO):
                                    nc.tensor.matmul(
                                        psv[:, half, :],
                                        xt[:, ko, sub * P:(sub + 1) * P],
                                        wv_sb[:, ko, :],
                                        start=(ko == 0), stop=(ko == KO - 1))
                            tci = ti * (TQ // P) + pair * 2
                            nc.vector.tensor_copy(
                                vt[:, tci:tci + 2, :], psv[:])
                        if b == 0 and ti == 1:
                            # big out-projection weight load: needed much
                            # later, stream it behind the hot loads
                            nc.sync.dma_start(wout_sb[:], wout.ap())

                    def attn_tile(hh, ta):
                        ps_o = pso.tile([P, TA], f32, tag="o")
                        ps_sum = pssum.tile([1, TA], f32, tag="sum")
                        smax = (ta + 1) * (TA // P) - 1
                        for s in range(smax + 1):
                            diag = s >= ta * (TA // P)
                            t_lo = (s - ta * (TA // P)) * P if diag else 0
                            w = slice(t_lo, TA)
                            qsl = slice(ta * TA + t_lo, (ta + 1) * TA)
                            ps_sc = pssc.tile([P, TA], f32, tag="sc")
                            nc.tensor.matmul(
                                ps_sc[:, w], kT[hh][:, s * P:(s + 1) * P],
                                qT[hh][:, qsl], start=True, stop=True)
                            pt = ptp.tile([P, TA], bf16, tag="pt")
                            nc.scalar.activation(
                                pt[:, w], ps_sc[:, w],
                                mybir.ActivationFunctionType.Exp,
                                scale=SCALE)
                            if diag:  # mask the 128x128 triangle
                                nc.gpsimd.tensor_mul(
                                    pt[:, t_lo:t_lo + P],
                                    pt[:, t_lo:t_lo + P], mask_sb[:])
                            first, last = (s == 0), (s == smax)
                            nc.tensor.matmul(ps_o[:, w],
                                             vt[:, s, hh * D:(hh + 1) * D],
                                             pt[:, w], start=first, stop=last)
                            nc.tensor.matmul(ps_sum[:, w], ones_col[:],
                                             pt[:, w], start=first, stop=last)
                        recf = sp.tile([1, TA], f32, tag="recf")
                        nc.vector.reciprocal_approx_fast(recf[:], ps_sum[:])
                        rec = sp.tile([1, TA], f32r, tag="rec")
                        nc.vector.tensor_copy(rec[:], recf[:])
                        ps_bc = pssc.tile([P, TA], f32, tag="sc")
                        nc.tensor.matmul(ps_bc[:], ones_rowr[:], rec[:],
                                         start=True, stop=True)
                        bc_sb = sp.tile([P, TA], f32, tag="bc_sb")
                        nc.vector.tensor_copy(bc_sb[:], ps_bc[:])
                        # write oT pre-shuffled for the out-projection:
                        # oT[p, j*128+u] = O^T[p, t=u*16+j]
                        oview = oT[b][hh].rearrange(
                            "p (j u) -> p u j", j=KO)[
                            :, (TA // 16) * ta:(TA // 16) * (ta + 1), :]
                        nc.vector.tensor_mul(
                            oview,
                            ps_o[:].rearrange("p (u j) -> p u j", j=KO),
                            bc_sb[:].rearrange("p (u j) -> p u j", j=KO))

                    def outproj(hh):
                        # out-projection for this head: psy shares the "o"
                        # PSUM slots with attention output tiles
                        for cpi in range(C // TC):
                            csl = slice(cpi * TC, (cpi + 1) * TC)
                            psy = pso.tile([P, TC], f32, tag="o")
                            for j in range(KO):
                                nc.tensor.matmul(psy[:],
                                                 oT[b][hh][:, j * P:(j + 1) * P],
                                                 wout_sb[:, j, csl],
                                                 start=(j == 0),
                                                 stop=(j == KO - 1))
                            ysb = sp.tile([P, TC], f32, tag="ysb")
                            nc.scalar.copy(ysb[:], psy[:])
                            nc.sync.dma_start(
                                y.ap()[b, hh * D:(hh + 1) * D, csl], ysb[:])

                    if b == 0:
                        # dummy matmuls warm the PE clock gate while the
                        # first DMAs are still in flight
                        ps_w = pssc.tile([P, TC], f32, tag="sc")
                        for _ in range(12):
                            nc.tensor.matmul(ps_w[:], warm[:, 0:P], warm[:],
                                             start=True, stop=True)

                    # attention tile `ta` only needs projected tokens
                    # t < (ta+1)*512, i.e. chunks 0..ta: interleave so the
                    # scalar-engine exp overlaps projection matmuls; each
                    # head's out-projection follows its last attention tile
                    # so the kernel tail overlaps the other stream
                    for ti in range(NT):
                        qkv_chunk(ti)
                        for hh in range(HPC):
                            attn_tile(hh, ti)
                            if ti == NT - 1:
                                outproj(hh)

    nc.compile()
    return nc


_NC = None


def _get_nc():
    global _NC
    if _NC is None:
        _NC = _build()
    return _NC


def _host_tables():
    pos = np.arange(T, dtype=np.float32)[:, None]
    div = np.exp(np.arange(0, 2 * HALF, 2, dtype=np.float32)
                 * np.float32(-math.log(ROPE_BASE) / (2 * HALF)))
    ang = pos * div[None, :]
    cosv = np.cos(ang).astype(np.float32)   # [T, HALF]
    sinv = np.sin(ang).astype(np.float32)
    cosT = np.ascontiguousarray(cosv.T)     # [HALF, T]
    sinT = np.ascontiguousarray(sinv.T)
    cs2 = np.ascontiguousarray(
        np.concatenate([cosT, cosT], axis=0)).astype(nbf16)  # [P, T]
    sn2 = np.ascontiguousarray(
        np.concatenate([-sinT, sinT], axis=0)).astype(nbf16)  # [P,T] -sin;+sin
    # triangle mask M[s, w] = 1 iff s <= w
    ww = np.arange(P)[None, :]
    ss = np.arange(P)[:, None]
    maskM = (ss <= ww).astype(nbf16)
    return cs2, sn2, maskM


def _make_in_maps(x, Wqkv, Wout):
    x = np.asarray(x, dtype=np.float32)
    Wqkv = np.asarray(Wqkv, dtype=np.float32)
    Wout = np.asarray(Wout, dtype=np.float32)
    assert x.shape == (B, T, C) and Wqkv.shape == (C, 3 * C) \
        and Wout.shape == (C, C)

    cs2, sn2, maskM = _host_tables()
    # xTt[b, ti, p, ko, u] = x[b, ti*TQ+u, ko*128+p]
    xTt = np.ascontiguousarray(
        x.reshape(B, NT, TQ, KO, P).transpose(0, 1, 4, 3, 2).astype(nbf16))
    # wout[p, j, n] = Wout[j*128+p, n]
    woutT = np.ascontiguousarray(
        Wout.reshape(KO, P, C).transpose(1, 0, 2).astype(nbf16))

    in_maps = []
    for core in range(NCORES):
        h0 = core * HPC
        cols = slice(h0 * D, (h0 + HPC) * D)
        ws = []
        for part in range(3):
            w = Wqkv[:, part * C:(part + 1) * C][:, cols]  # [C, HPC*D]
            ws.append(np.ascontiguousarray(
                w.reshape(KO, P, HPC * D).transpose(1, 0, 2).astype(nbf16)))
        in_maps.append({
            "xTt": xTt,
            "wq": ws[0], "wk": ws[1], "wv": ws[2],
            "wout": woutT,
            "cs2": cs2, "sn2": sn2, "maskM": maskM,
        })
    return in_maps


def _run(x, Wqkv, Wout, trace=False):
    nc = _get_nc()
    in_maps = _make_in_maps(x, Wqkv, Wout)
    res = run_bass_kernel_spmd(nc, in_maps, core_ids=list(range(NCORES)),
                               trace=trace)
    out = np.empty((B, T, C), dtype=np.float32)
    for core in range(NCORES):
        out[:, core * HPC * D:(core + 1) * HPC * D, :] = \
            res.results[core]["y"]
    return out, res


def kernel(x, Wqkv, Wout):
    out, _ = _run(x, Wqkv, Wout)
    return out
